# revision 3
# baseline (speedup 1.0000x reference)
"""Trainium2 Bass kernel for nn_Encoder_7413113553686.

Key algebraic fact: the reference loops `out = _guidance(x0, ...)` with the
SAME x0 every iteration, so only the last block (i = 20) matters.

Work split host/device:
  host   : patch-embed conv + LayerNorm + pos  ->  x0   (0.4 GFLOP, 1.2%)
           final gather: pair-sum + x0 residual + proj bias
  device : q/k/v GEMMs, attention softmax, projection partial sums

Sharding over 8 cores: core c = (b, g), b = c>>1 (batch), g = c&1
(head-group: heads 4g..4g+3).  Each core computes its 4 heads and the
projection partial from its 384 head-dims; the host sums the pair.

Everything on device is bf16 (matmul operands + DMA payloads, ~3.9 MB per
core); psum accumulation stays fp32.  Final rel err ~9.5e-5.

Device schedule (in rough execution order; the PE is the critical engine):
  front  : q/k GEMMs for heads 0+1, k-chunk-major over six open PSUM
           accumulators so each x0T chunk is consumed as its DMA lands;
           inputs split over the three DMA-issue queues in need order
  v      : token-major V with an appended ones-column (the softmax
           denominator falls out of the attn@V matmul); tiles 0-1 up
           front, tiles 2-7 emitted as head-0 PE filler
  attn   : per head, per key-chunk: scores -> ACT exp straight from PSUM
           (bf16 out) -> attn@V into split [97,512] accumulators. Scores
           run two chunks ahead and attn@V one behind, so the ~1.15us
           ACT exp latency never stalls the PE.  Heads 1-3's q/k GEMM
           halves are interleaved as PE filler in the preceding head.
           The softmax reciprocal runs deferred: denominator row spread
           over 128 partitions via reshape-DMA, full-width reciprocal,
           DMA back, gpsimd broadcast; the unnormalized output is
           copy-evicted immediately so the PSUM frees without waiting.
  proj   : packed K=128 over a PERMUTED head-concat (heads 0/3/2 at
           partition 0 of the three packed tiles, head 1 split across
           the [96:128] remainders) so the last head's normalize is a
           single full-width multiply; proj m-tiles on head-3-free
           K-chunks pre-start inside head 3 and right after it, hiding
           the reciprocal-chain latency.
"""

import os
import sys

import numpy as np
import ml_dtypes

for _p in ("/opt/trn_rl_repo",):
    if os.path.isdir(_p) and _p not in sys.path:
        sys.path.insert(0, _p)

from concourse import bacc, bass, mybir, tile  # noqa: E402
from concourse.bass_utils import run_bass_kernel_spmd  # noqa: E402

F32 = mybir.dt.float32
BF16 = mybir.dt.bfloat16
NPBF = ml_dtypes.bfloat16

B, D, N, NH, HD = 4, 768, 1024, 8, 96
SCALE = float(HD) ** -0.5
LAYER = 20
AF = mybir.ActivationFunctionType

# Permuted packed layout for the projection K-chunks: heads 0/3/2 sit at
# partition 0 of oTn tiles 0/1/2 (single-block evictions; head 3's gates the
# tail), head 1 is split across the three [96:128] regions.  The host
# permutes the proj_w rows to match (_PERM).
_PERM = (list(range(0, 128)) + list(range(288, 384)) + list(range(128, 160))
         + list(range(192, 288)) + list(range(160, 192)))
# head -> [(src_p0, tile_idx, dst_p0, len)]
_EVICT = {
    0: [(0, 0, 0, 96)],
    1: [(0, 0, 96, 32), (32, 1, 96, 32), (64, 2, 96, 32)],
    2: [(0, 2, 0, 96)],
    3: [(0, 1, 0, 96)],
}


def _pack6(a):
    # [768, 384] -> [128, 6*384]: k-chunks of 128 rows side by side
    return np.ascontiguousarray(
        np.concatenate([a[k * 128:(k + 1) * 128] for k in range(6)], 1)
    ).astype(NPBF)


def _pack3(a):
    # [384, 768] -> [128, 3*768]
    return np.ascontiguousarray(
        np.concatenate([a[i * 128:(i + 1) * 128] for i in range(3)], 1)
    ).astype(NPBF)


def _body(nc, tc, io, out_dram):
    mm = nc.tensor.matmul

    import contextlib
    _persist_ctx = contextlib.ExitStack()
    persist = _persist_ctx.enter_context(tc.tile_pool(name="persist", bufs=1))

    def ptile(name, shape, dtype=F32):
        return persist.tile(shape, dtype, tag=name, name=name)

    # ---------------- input DMAs ----------------
    # three issue queues; first-needed bytes first: x0T chunk 0, then the
    # q/k weights, then the remaining x0T chunks, then v/proj weights
    x0T_all = ptile("x0T", [128, 6 * N], BF16)
    x0T = [x0T_all[:, k * N:(k + 1) * N] for k in range(6)]
    qw_t = ptile("qw", [128, 2304], BF16)
    kw_t = ptile("kw", [128, 2304], BF16)
    vw_t = ptile("vw", [128, 2304], BF16)
    pw_t = ptile("pwt", [128, 2304], BF16)

    def dma_in(eng, dst, name, k=None):
        srcap = io[name][:, :] if k is None else io[name][k * 128:(k + 1) * 128, :]
        eng.dma_start(out=dst, in_=srcap)

    dma_in(nc.sync, x0T[0], "x0T", 0)
    for p, pname in enumerate(("qwa", "qwb", "qwc")):
        nc.gpsimd.dma_start(out=qw_t[:, p * 768:(p + 1) * 768],
                            in_=io[pname][:, :])
    for p, pname in enumerate(("kwa", "kwb", "kwc")):
        nc.scalar.dma_start(out=kw_t[:, p * 768:(p + 1) * 768],
                            in_=io[pname][:, :])
    dma_in(nc.sync, x0T[1], "x0T", 1)
    dma_in(nc.sync, x0T[4], "x0T", 4)
    dma_in(nc.sync, vw_t[:, :], "vw")
    dma_in(nc.gpsimd, x0T[2], "x0T", 2)
    dma_in(nc.scalar, x0T[3], "x0T", 3)
    dma_in(nc.gpsimd, x0T[5], "x0T", 5)
    dma_in(nc.sync, pw_t[:, :], "pw")
    _KORDER = [0, 1, 4, 2, 3, 5]

    qw = [qw_t[:, k * 384:(k + 1) * 384] for k in range(6)]
    kw = [kw_t[:, k * 384:(k + 1) * 384] for k in range(6)]
    vw = [vw_t[:, k * 384:(k + 1) * 384] for k in range(6)]
    pwc = [pw_t[:, i * D:(i + 1) * D] for i in range(3)]

    # persistent activations
    qT = [ptile(f"qT{h}", [96, N], BF16) for h in range(4)]
    kT = [ptile(f"kT{h}", [96, N], BF16) for h in range(4)]
    v3_all = ptile("v3", [128, 8 * 4 * 97], BF16)
    v3 = [v3_all[:, m * 4 * 97:(m + 1) * 4 * 97] for m in range(8)]
    oTf = [ptile(f"oTf{h}", [96, N]) for h in range(4)]
    oTn = [ptile(f"oTn{i}", [128, N], BF16) for i in range(3)]
    srow = ptile("srow", [128, N])  # head h uses partition h*32
    rb = [ptile(f"rb{h}", [96, N]) for h in range(4)]

    # warm the Exp ACT table during the DMA ramp
    warm = ptile("warm", [1, 2])
    nc.gpsimd.memset(warm[:, :], 0.0)
    nc.scalar.activation(warm[:, 1:2], warm[:, 0:1], AF.Exp)
    # ones columns for the softmax-denominator trick, one strided DMA
    v4d = v3_all.rearrange("p (m h d) -> p m h d", m=8, h=4)
    nc.gpsimd.dma_start(out=v4d[:, :, :, 96:97], in_=io["vones"][:, :, :, None])

    with (
        tc.tile_pool(name="ps", bufs=2, space="PSUM") as ps,
        tc.tile_pool(name="wk", bufs=2) as wk,
    ):
        # ---------------- front: q/k GEMMs for heads 0+1, k-major -----------
        # six accumulators (two full A-tiles + four B-halves); each x0T
        # chunk is fully consumed the moment it lands
        fA = [ps.tile([96, N], F32, tag="A", name=f"fA{i}") for i in range(2)]
        fB = {}
        for hw in range(2):
            for n in range(2):
                fB[(hw, n)] = ps.tile([96, 512], F32, tag="B",
                                      name=f"fB{hw}{n}", bufs=4)
        for ki, k in enumerate(_KORDER):
            st, sp = (ki == 0), (ki == 5)
            for n in range(2):
                sl = bass.ts(n, 512)
                mm(fA[0][:, sl], qw[k][:, 0:96], x0T[k][:, sl],
                   start=st, stop=sp)
            for n in range(2):
                sl = bass.ts(n, 512)
                mm(fA[1][:, sl], kw[k][:, 0:96], x0T[k][:, sl],
                   start=st, stop=sp)
            for hw in range(2):
                wl = qw if hw == 0 else kw
                for n in range(2):
                    mm(fB[(hw, n)][:, :], wl[k][:, 96:192],
                       x0T[k][:, bass.ts(n, 512)], start=st, stop=sp)
        with nc.allow_low_precision(reason="qk evict to bf16"):
            nc.vector.tensor_copy(qT[0][:, :], fA[0][:, :])
            nc.vector.tensor_copy(kT[0][:, :], fA[1][:, :])
            for hw in range(2):
                dst = qT if hw == 0 else kT
                for n in range(2):
                    nc.vector.tensor_copy(dst[1][:, bass.ts(n, 512)],
                                          fB[(hw, n)][:, :])

        # ---------------- late q/k GEMM halves (heads 2,3 as fillers) -------
        def emit_qk_half(h, which, n):
            hs = slice(h * 96, (h + 1) * 96)
            sl = bass.ts(n, 512)
            wl, dst = (qw, qT) if which == 0 else (kw, kT)
            pq = ps.tile([96, 512], F32, tag="B", name=f"pq{h}_{which}_{n}",
                         bufs=4)
            for k in range(6):
                mm(pq[:, :], wl[k][:, hs], x0T[k][:, sl],
                   start=(k == 0), stop=(k == 5))
            with nc.allow_low_precision(reason="qk evict to bf16"):
                nc.vector.tensor_copy(dst[h][:, sl], pq[:, :])

        # ---------------- V = x0 @ vw (token-major + ones col) --------------
        def emit_v(m):
            pv = ps.tile([128, 384], F32, tag="B", name=f"pv{m}", bufs=4)
            for k in range(6):
                mm(pv[:, :], x0T[k][:, m * 128:(m + 1) * 128], vw[k][:, :],
                   start=(k == 0), stop=(k == 5))
            v3m = v3[m].rearrange("p (h d) -> p h d", h=4)
            with nc.allow_low_precision(reason="v evict to bf16"):
                nc.vector.tensor_copy(
                    v3m[:, :, 0:96], pv.rearrange("p (h d) -> p h d", h=4))

        emit_v(0)
        emit_v(1)

        # ---------------- attention ----------------
        def emit_sc(h, m):
            pss = ps.tile([128, N], F32, tag="A", name=f"pss{h}_{m}")
            for n in range(2):
                sl = bass.ts(n, 512)
                mm(pss[:, sl], kT[h][:, m * 128:(m + 1) * 128], qT[h][:, sl],
                   start=True, stop=True)
            ex = wk.tile([128, N], BF16, tag="ex", name=f"ex{h}_{m}", bufs=4)
            nc.scalar.activation(ex[:, :], pss[:, :], AF.Exp)
            return ex

        def emit_av(h, m, ex, po):
            v3v = v3[m].rearrange("p (h d) -> p h d", h=4)
            for n in range(2):
                sl = bass.ts(n, 512)
                mm(po[n][:, :], v3v[:, h, :], ex[:, sl],
                   start=(m == 0), stop=(m == 7))

        def emit_po_evict(h):
            # denominator row out first (gates the reciprocal chain), then a
            # fast unnormalized copy-evict so the PSUM halves free without
            # waiting for the reciprocal
            r0 = srow[h * 32:h * 32 + 1, :]
            nc.vector.tensor_copy(r0[:, 0:512], po_t[h][0][96:97, :])
            if h == 3:
                nc.scalar.copy(r0[:, 512:1024], po_t[h][1][96:97, :])
            else:
                nc.vector.tensor_copy(r0[:, 512:1024], po_t[h][1][96:97, :])
            for n in range(2):
                nc.vector.tensor_copy(oTf[h][:, bass.ts(n, 512)],
                                      po_t[h][n][0:96, :])

        def emit_recip(h):
            s_pk = wk.tile([128, 8], F32, tag="spk", name=f"spk{h}")
            nc.sync.dma_start(out=s_pk[:, :], in_=srow[h * 32:h * 32 + 1, :])
            r_pk = wk.tile([128, 8], F32, tag="rpk", name=f"rpk{h}")
            nc.vector.reciprocal(r_pk[:, :], s_pk[:, :])
            recip = wk.tile([1, N], F32, tag="rc", name=f"rc{h}")
            nc.sync.dma_start(out=recip[:, :], in_=r_pk[:, :])
            nc.gpsimd.partition_broadcast(rb[h][:, :], recip[:, :])

        def emit_norm(h):
            # normalize into the permuted packed oTn tiles
            with nc.allow_low_precision(reason="attn out normalize to bf16"):
                for (s, t, d_, ln) in _EVICT[h]:
                    nc.vector.tensor_mul(oTn[t][d_:d_ + ln, :],
                                         oTf[h][s:s + ln, :],
                                         rb[h][s:s + ln, :])

        # ---------------- proj helpers (packed, permuted K-chunks) ----------
        def emit_pp_start(m, kcs):
            pp = ps.tile([128, N], F32, tag="A", name=f"pp{m}")
            for j, i in enumerate(kcs):
                for n in range(2):
                    sl = bass.ts(n, 512)
                    mm(pp[:, sl], pwc[i][:, m * 128:(m + 1) * 128],
                       oTn[i][:, sl], start=(j == 0), stop=False)
            return pp

        def emit_pp_finish(m, pp, kcs, eng):
            for j, i in enumerate(kcs):
                for n in range(2):
                    sl = bass.ts(n, 512)
                    mm(pp[:, sl], pwc[i][:, m * 128:(m + 1) * 128],
                       oTn[i][:, sl], start=False, stop=(j == len(kcs) - 1))
            ou = wk.tile([128, N], BF16, tag="out", name=f"ou{m}")
            with nc.allow_low_precision(reason="partial proj sums to bf16"):
                if eng == 0:
                    nc.vector.tensor_copy(ou[:, :], pp[:, :])
                else:
                    nc.scalar.copy(ou[:, :], pp[:, :])
            dmaeng = nc.sync if eng == 0 else nc.gpsimd
            dmaeng.dma_start(out=out_dram[m * 128:(m + 1) * 128, :],
                             in_=ou[:, :])

        po_t = {}
        for h in range(4):
            po_t[h] = (ps.tile([97, 512], F32, tag="B", name=f"po{h}_0",
                               bufs=4),
                       ps.tile([97, 512], F32, tag="B", name=f"po{h}_1",
                               bufs=4))
            exs = {}
            exs[0] = emit_sc(h, 0)
            exs[1] = emit_sc(h, 1)
            for m in range(8):
                # PE fillers: head 0 chews the remaining V tiles, heads 1-2
                # the next head's q/k halves, head 3 the proj prestart
                if h == 0 and m < 6:
                    emit_v(m + 2)
                elif h in (1, 2) and m in (0, 2, 3, 5):
                    emit_qk_half(h + 1, *((0, 0) if m == 0 else
                                          (0, 1) if m == 2 else
                                          (1, 0) if m == 3 else (1, 1)))
                elif h == 3 and m == 3:
                    # prestart proj m=0 (K-chunks 0/2, head-3-free) in the
                    # two spare B half-slots as head-3 PE filler
                    pp0h = []
                    for n in range(2):
                        sl = bass.ts(n, 512)
                        pph = ps.tile([128, 512], F32, tag="B",
                                      name=f"pp0_{n}", bufs=4)
                        for j, i in enumerate([0, 2]):
                            mm(pph[:, :], pwc[i][:, 0:128],
                               oTn[i][:, sl], start=(j == 0), stop=False)
                        pp0h.append(pph)
                if m < 6:
                    exs[m + 2] = emit_sc(h, m + 2)
                if m >= 1:
                    emit_av(h, m - 1, exs[m - 1], po_t[h])
                    del exs[m - 1]
            emit_av(h, 7, exs[7], po_t[h])
            del exs[7]
            emit_po_evict(h)
            emit_recip(h)
            emit_norm(h)
            if h == 3:
                # more prestart right after the last attn@V: m=1 in the A
                # slots (pss rotation is over), m=2 in B once po(3) evicts
                pp_pre1 = emit_pp_start(1, [0, 2])
                pp2h = []
                for n in range(2):
                    sl = bass.ts(n, 512)
                    pph = ps.tile([128, 512], F32, tag="B",
                                  name=f"pp2_{n}", bufs=4)
                    for j, i in enumerate([0, 2]):
                        mm(pph[:, :], pwc[i][:, 2 * 128:3 * 128],
                           oTn[i][:, sl], start=(j == 0), stop=False)
                    pp2h.append(pph)

        # ---------------- proj finish ----------------
        def finish_halves(m, pphs, ou_name, eng):
            ou = wk.tile([128, N], BF16, tag="out", name=ou_name)
            for n in range(2):
                sl = bass.ts(n, 512)
                mm(pphs[n][:, :], pwc[1][:, m * 128:(m + 1) * 128],
                   oTn[1][:, sl], start=False, stop=True)
                with nc.allow_low_precision(reason="partial proj to bf16"):
                    if n == 0:
                        nc.vector.tensor_copy(ou[:, sl], pphs[n][:, :])
                    else:
                        nc.scalar.copy(ou[:, sl], pphs[n][:, :])
            dmaeng = nc.sync if eng == 0 else nc.gpsimd
            dmaeng.dma_start(out=out_dram[m * 128:(m + 1) * 128, :],
                             in_=ou[:, :])

        finish_halves(0, pp0h, "ou0", 0)
        emit_pp_finish(1, pp_pre1, [1], 1)
        finish_halves(2, pp2h, "ou2", 0)
        for m in range(3, 6):
            pp = emit_pp_start(m, [0, 2])
            emit_pp_finish(m, pp, [1], m % 2)

    _persist_ctx.close()


def _build_nc():
    nc = bacc.Bacc("TRN2", target_bir_lowering=False, debug=False,
                   enable_asserts=False)
    io = {}
    for name, shape in (("x0T", [D, N]),
                        ("qwa", [128, 768]), ("qwb", [128, 768]),
                        ("qwc", [128, 768]),
                        ("kwa", [128, 768]), ("kwb", [128, 768]),
                        ("kwc", [128, 768]), ("vw", [128, 2304]),
                        ("pw", [128, 2304]),
                        ("vones", [128, 8, 4])):
        io[name] = nc.dram_tensor(name, shape, BF16, kind="ExternalInput").ap()
    out_dram = nc.dram_tensor("o", [D, N], BF16, kind="ExternalOutput").ap()
    with tile.TileContext(nc) as tc:
        _body(nc, tc, io, out_dram)
    nc.compile()
    return nc


_NC_CACHE = {}


def _get_nc():
    if "nc" not in _NC_CACHE:
        _NC_CACHE["nc"] = _build_nc()
    return _NC_CACHE["nc"]


_HOST_STATE = {}


def _prep_in_maps(sam, conv_w, conv_b, ln_g, ln_b, pos, q_w, kv_w, proj_w,
                  proj_b):
    f = np.float32
    sam = np.asarray(sam, f)
    qwL = (np.asarray(q_w[LAYER], f) * SCALE).astype(f)
    kvL = np.asarray(kv_w[LAYER], f)
    kwL, vwL = kvL[:, :D], kvL[:, D:]
    pwL = np.asarray(proj_w[LAYER], f)
    pbL = np.asarray(proj_b[LAYER], f)

    # host patch-embed conv + LN + pos  (exact fp32; 0.4 GFLOP)
    W2 = np.asarray(conv_w, f).reshape(D, 64).T
    cb = np.asarray(conv_b, f)
    g_ = np.asarray(ln_g, f)
    b_ = np.asarray(ln_b, f)
    posf = np.asarray(pos, f)
    x0 = np.empty((B, N, D), f)
    for b in range(B):
        img = sam[b, 0]
        patches = img.reshape(32, 8, 32, 8).transpose(0, 2, 1, 3).reshape(N, 64)
        x = patches @ W2 + cb[None, :]
        mu = x.mean(-1, keepdims=True)
        var = ((x - mu) ** 2).mean(-1, keepdims=True)
        x0[b] = (x - mu) / np.sqrt(var + 1e-5) * g_ + b_ + posf
    _HOST_STATE["x0"] = x0
    _HOST_STATE["pb"] = pbL

    in_maps = []
    for c in range(8):
        b, g = c >> 1, c & 1
        sl = slice(g * 384, (g + 1) * 384)
        in_maps.append({
            "x0T": np.ascontiguousarray(x0[b].T).astype(NPBF),
            "qwa": np.ascontiguousarray(_pack6(qwL[:, sl])[:, 0:768]),
            "qwb": np.ascontiguousarray(_pack6(qwL[:, sl])[:, 768:1536]),
            "qwc": np.ascontiguousarray(_pack6(qwL[:, sl])[:, 1536:2304]),
            "kwa": np.ascontiguousarray(_pack6(kwL[:, sl])[:, 0:768]),
            "kwb": np.ascontiguousarray(_pack6(kwL[:, sl])[:, 768:1536]),
            "kwc": np.ascontiguousarray(_pack6(kwL[:, sl])[:, 1536:2304]),
            "vw": _pack6(vwL[:, sl]),
            "pw": _pack3(pwL[sl, :][_PERM, :]),
            "vones": np.ones((128, 8, 4), np.float32).astype(NPBF),
        })
    return in_maps


def _gather(results):
    x0 = _HOST_STATE["x0"]
    pb = _HOST_STATE["pb"]
    outs = [np.asarray(r["o"]).astype(np.float32) for r in results]
    full = np.stack([(outs[2 * b] + outs[2 * b + 1]).T + x0[b] + pb[None, :]
                     for b in range(B)])
    return np.ascontiguousarray(full.astype(np.float32))


def kernel(sam, conv_w, conv_b, ln_g, ln_b, pos, q_w, kv_w, proj_w, proj_b,
           **_unused):
    nc = _get_nc()
    in_maps = _prep_in_maps(sam, conv_w, conv_b, ln_g, ln_b, pos, q_w, kv_w,
                            proj_w, proj_b)
    res = run_bass_kernel_spmd(nc, in_maps, core_ids=list(range(8)))
    return _gather(res.results)


if __name__ == "__main__":
    sys.path.insert(0, os.path.dirname(os.path.abspath(__file__)))
    import reference as R

    inputs = {k: np.asarray(v) for k, v in R.setup_inputs().items()}
    expected = np.asarray(R.reference(**inputs))
    actual = kernel(**inputs)
    rel = np.linalg.norm(actual - expected) / np.linalg.norm(expected)
    print("Relative error:", rel)


# revision 5
# speedup vs baseline: 1.0134x; 1.0134x over previous
"""Trainium2 Bass kernel for nn_Encoder_7413113553686.

Key algebraic fact: the reference loops `out = _guidance(x0, ...)` with the
SAME x0 every iteration, so only the last block (i = 20) matters.

Work split host/device:
  host   : patch-embed conv + LayerNorm + pos  ->  x0   (0.4 GFLOP, 1.2%)
           final gather: pair-sum + x0 residual + proj bias
  device : q/k/v GEMMs, attention softmax, projection partial sums

Sharding over 8 cores: core c = (b, g), b = c>>1 (batch), g = c&1
(head-group: heads 4g..4g+3).  Each core computes its 4 heads and the
projection partial from its 384 head-dims; the host sums the pair.

Everything on device is bf16 (matmul operands + DMA payloads, ~3.9 MB per
core); psum accumulation stays fp32.  Final rel err ~9.5e-5.

Device schedule (in rough execution order; the PE is the critical engine):
  front  : q/k GEMMs for heads 0+1, k-chunk-major over six open PSUM
           accumulators so each x0T chunk is consumed as its DMA lands;
           inputs split over the three DMA-issue queues in need order
  v      : token-major V with an appended ones-column (the softmax
           denominator falls out of the attn@V matmul); tiles 0-1 up
           front, tiles 2-7 emitted as head-0 PE filler
  attn   : per head, per key-chunk: scores -> ACT exp straight from PSUM
           (bf16 out) -> attn@V into split [97,512] accumulators. Scores
           run two chunks ahead and attn@V one behind, so the ~1.15us
           ACT exp latency never stalls the PE.  Heads 1-3's q/k GEMM
           halves are interleaved as PE filler in the preceding head.
           The softmax reciprocal runs deferred: denominator row spread
           over 128 partitions via reshape-DMA, full-width reciprocal,
           DMA back, gpsimd broadcast; the unnormalized output is
           copy-evicted immediately so the PSUM frees without waiting.
  proj   : packed K=128 over a PERMUTED head-concat (heads 0/3/2 at
           partition 0 of the three packed tiles, head 1 split across
           the [96:128] remainders) so the last head's normalize is a
           single full-width multiply; proj m-tiles on head-3-free
           K-chunks pre-start inside head 3 and right after it, hiding
           the reciprocal-chain latency.
"""

import os
import sys

import numpy as np
import ml_dtypes

for _p in ("/opt/trn_rl_repo",):
    if os.path.isdir(_p) and _p not in sys.path:
        sys.path.insert(0, _p)

from concourse import bacc, bass, mybir, tile  # noqa: E402
from concourse.bass_utils import run_bass_kernel_spmd  # noqa: E402

F32 = mybir.dt.float32
BF16 = mybir.dt.bfloat16
NPBF = ml_dtypes.bfloat16

B, D, N, NH, HD = 4, 768, 1024, 8, 96
SCALE = float(HD) ** -0.5
LAYER = 20
AF = mybir.ActivationFunctionType

# Permuted packed layout for the projection K-chunks: heads 0/3/2 sit at
# partition 0 of oTn tiles 0/1/2 (single-block evictions; head 3's gates the
# tail), head 1 is split across the three [96:128] regions.  The host
# permutes the proj_w rows to match (_PERM).
_PERM = (list(range(0, 128)) + list(range(288, 384)) + list(range(128, 160))
         + list(range(192, 288)) + list(range(160, 192)))
# head -> [(src_p0, tile_idx, dst_p0, len)]
_EVICT = {
    0: [(0, 0, 0, 96)],
    1: [(0, 0, 96, 32), (32, 1, 96, 32), (64, 2, 96, 32)],
    2: [(0, 2, 0, 96)],
    3: [(0, 1, 0, 96)],
}


def _pack6(a):
    # [768, 384] -> [128, 6*384]: k-chunks of 128 rows side by side
    return np.ascontiguousarray(
        np.concatenate([a[k * 128:(k + 1) * 128] for k in range(6)], 1)
    ).astype(NPBF)


def _pack3(a):
    # [384, 768] -> [128, 3*768]
    return np.ascontiguousarray(
        np.concatenate([a[i * 128:(i + 1) * 128] for i in range(3)], 1)
    ).astype(NPBF)


def _body(nc, tc, io, out_dram):
    mm = nc.tensor.matmul

    import contextlib
    _persist_ctx = contextlib.ExitStack()
    persist = _persist_ctx.enter_context(tc.tile_pool(name="persist", bufs=1))

    def ptile(name, shape, dtype=F32):
        return persist.tile(shape, dtype, tag=name, name=name)

    # ---------------- input DMAs ----------------
    # three issue queues; first-needed bytes first: x0T chunk 0, then the
    # q/k weights, then the remaining x0T chunks, then v/proj weights
    x0T_all = ptile("x0T", [128, 6 * N], BF16)
    x0T = [x0T_all[:, k * N:(k + 1) * N] for k in range(6)]
    qw_t = ptile("qw", [128, 2304], BF16)
    kw_t = ptile("kw", [128, 2304], BF16)
    vw_t = ptile("vw", [128, 2304], BF16)
    pw_t = ptile("pwt", [128, 2304], BF16)

    def dma_in(eng, dst, name, k=None):
        srcap = io[name][:, :] if k is None else io[name][k * 128:(k + 1) * 128, :]
        eng.dma_start(out=dst, in_=srcap)

    dma_in(nc.sync, x0T[0], "x0T", 0)
    for p, pname in enumerate(("qwa", "qwb", "qwc")):
        nc.gpsimd.dma_start(out=qw_t[:, p * 768:(p + 1) * 768],
                            in_=io[pname][:, :])
    for p, pname in enumerate(("kwa", "kwb", "kwc")):
        nc.scalar.dma_start(out=kw_t[:, p * 768:(p + 1) * 768],
                            in_=io[pname][:, :])
    dma_in(nc.sync, x0T[1], "x0T", 1)
    dma_in(nc.sync, x0T[4], "x0T", 4)
    dma_in(nc.sync, vw_t[:, :], "vw")
    dma_in(nc.gpsimd, x0T[2], "x0T", 2)
    dma_in(nc.scalar, x0T[3], "x0T", 3)
    dma_in(nc.gpsimd, x0T[5], "x0T", 5)
    dma_in(nc.sync, pw_t[:, :], "pw")
    _KORDER = [0, 1, 4, 2, 3, 5]

    qw = [qw_t[:, k * 384:(k + 1) * 384] for k in range(6)]
    kw = [kw_t[:, k * 384:(k + 1) * 384] for k in range(6)]
    vw = [vw_t[:, k * 384:(k + 1) * 384] for k in range(6)]
    pwc = [pw_t[:, i * D:(i + 1) * D] for i in range(3)]

    # persistent activations
    qT = [ptile(f"qT{h}", [96, N], BF16) for h in range(4)]
    kT = [ptile(f"kT{h}", [96, N], BF16) for h in range(4)]
    v3_all = ptile("v3", [128, 8 * 4 * 97], BF16)
    v3 = [v3_all[:, m * 4 * 97:(m + 1) * 4 * 97] for m in range(8)]
    oTf = [ptile(f"oTf{h}", [96, N]) for h in range(4)]
    oTn = [ptile(f"oTn{i}", [128, N], BF16) for i in range(3)]
    srow = ptile("srow", [128, N])  # head h uses partition h*32
    rb = [ptile(f"rb{h}", [96, N]) for h in range(4)]

    # warm the Exp ACT table during the DMA ramp
    warm = ptile("warm", [1, 2])
    nc.gpsimd.memset(warm[:, :], 0.0)
    nc.scalar.activation(warm[:, 1:2], warm[:, 0:1], AF.Exp)
    # ones columns for the softmax-denominator trick, one strided DMA
    v4d = v3_all.rearrange("p (m h d) -> p m h d", m=8, h=4)
    nc.gpsimd.dma_start(out=v4d[:, :, :, 96:97], in_=io["vones"][:, :, :, None])

    with (
        tc.tile_pool(name="ps", bufs=2, space="PSUM") as ps,
        tc.tile_pool(name="wk", bufs=2) as wk,
    ):
        # ---------------- front: q/k GEMMs for heads 0+1, k-major -----------
        # six accumulators (two full A-tiles + four B-halves); each x0T
        # chunk is fully consumed the moment it lands
        fA = [ps.tile([96, N], F32, tag="A", name=f"fA{i}") for i in range(2)]
        fB = {}
        for hw in range(2):
            for n in range(2):
                fB[(hw, n)] = ps.tile([96, 512], F32, tag="B",
                                      name=f"fB{hw}{n}", bufs=4)
        for ki, k in enumerate(_KORDER):
            st, sp = (ki == 0), (ki == 5)
            for n in range(2):
                sl = bass.ts(n, 512)
                mm(fA[0][:, sl], qw[k][:, 0:96], x0T[k][:, sl],
                   start=st, stop=sp)
            for n in range(2):
                sl = bass.ts(n, 512)
                mm(fA[1][:, sl], kw[k][:, 0:96], x0T[k][:, sl],
                   start=st, stop=sp)
            for hw in range(2):
                wl = qw if hw == 0 else kw
                for n in range(2):
                    mm(fB[(hw, n)][:, :], wl[k][:, 96:192],
                       x0T[k][:, bass.ts(n, 512)], start=st, stop=sp)
        with nc.allow_low_precision(reason="qk evict to bf16"):
            nc.vector.tensor_copy(qT[0][:, :], fA[0][:, :])
            nc.vector.tensor_copy(kT[0][:, :], fA[1][:, :])
            for hw in range(2):
                dst = qT if hw == 0 else kT
                for n in range(2):
                    nc.vector.tensor_copy(dst[1][:, bass.ts(n, 512)],
                                          fB[(hw, n)][:, :])

        # ---------------- late q/k GEMM halves (heads 2,3 as fillers) -------
        def emit_qk_half(h, which, n):
            hs = slice(h * 96, (h + 1) * 96)
            sl = bass.ts(n, 512)
            wl, dst = (qw, qT) if which == 0 else (kw, kT)
            pq = ps.tile([96, 512], F32, tag="B", name=f"pq{h}_{which}_{n}",
                         bufs=4)
            for k in range(6):
                mm(pq[:, :], wl[k][:, hs], x0T[k][:, sl],
                   start=(k == 0), stop=(k == 5))
            with nc.allow_low_precision(reason="qk evict to bf16"):
                nc.vector.tensor_copy(dst[h][:, sl], pq[:, :])

        # ---------------- V = x0 @ vw (token-major + ones col) --------------
        def emit_v(m):
            pv = ps.tile([128, 384], F32, tag="B", name=f"pv{m}", bufs=4)
            for k in range(6):
                mm(pv[:, :], x0T[k][:, m * 128:(m + 1) * 128], vw[k][:, :],
                   start=(k == 0), stop=(k == 5))
            v3m = v3[m].rearrange("p (h d) -> p h d", h=4)
            with nc.allow_low_precision(reason="v evict to bf16"):
                nc.vector.tensor_copy(
                    v3m[:, :, 0:96], pv.rearrange("p (h d) -> p h d", h=4))

        emit_v(0)
        emit_v(1)

        # ---------------- attention ----------------
        def emit_sc(h, m):
            pss = ps.tile([128, N], F32, tag="A", name=f"pss{h}_{m}")
            for n in range(2):
                sl = bass.ts(n, 512)
                mm(pss[:, sl], kT[h][:, m * 128:(m + 1) * 128], qT[h][:, sl],
                   start=True, stop=True)
            ex = wk.tile([128, N], BF16, tag="ex", name=f"ex{h}_{m}", bufs=4)
            nc.scalar.activation(ex[:, :], pss[:, :], AF.Exp)
            return ex

        def emit_av(h, m, ex, po):
            v3v = v3[m].rearrange("p (h d) -> p h d", h=4)
            for n in range(2):
                sl = bass.ts(n, 512)
                mm(po[n][:, :], v3v[:, h, :], ex[:, sl],
                   start=(m == 0), stop=(m == 7))

        def emit_po_evict(h):
            # denominator row out first (gates the reciprocal chain), then a
            # fast unnormalized copy-evict so the PSUM halves free without
            # waiting for the reciprocal
            r0 = srow[h * 32:h * 32 + 1, :]
            nc.vector.tensor_copy(r0[:, 0:512], po_t[h][0][96:97, :])
            if h == 3:
                nc.scalar.copy(r0[:, 512:1024], po_t[h][1][96:97, :])
            else:
                nc.vector.tensor_copy(r0[:, 512:1024], po_t[h][1][96:97, :])
            for n in range(2):
                nc.vector.tensor_copy(oTf[h][:, bass.ts(n, 512)],
                                      po_t[h][n][0:96, :])

        def emit_recip(h):
            s_pk = wk.tile([128, 8], F32, tag="spk", name=f"spk{h}")
            nc.sync.dma_start(out=s_pk[:, :], in_=srow[h * 32:h * 32 + 1, :])
            r_pk = wk.tile([128, 8], F32, tag="rpk", name=f"rpk{h}")
            nc.vector.reciprocal(r_pk[:, :], s_pk[:, :])
            recip = wk.tile([1, N], F32, tag="rc", name=f"rc{h}")
            nc.sync.dma_start(out=recip[:, :], in_=r_pk[:, :])
            nc.gpsimd.partition_broadcast(rb[h][:, :], recip[:, :])

        def emit_norm(h):
            # normalize into the permuted packed oTn tiles
            with nc.allow_low_precision(reason="attn out normalize to bf16"):
                for (s, t, d_, ln) in _EVICT[h]:
                    nc.vector.tensor_mul(oTn[t][d_:d_ + ln, :],
                                         oTf[h][s:s + ln, :],
                                         rb[h][s:s + ln, :])

        # ---------------- proj helpers (packed, permuted K-chunks) ----------
        def emit_pp_start(m, kcs):
            pp = ps.tile([128, N], F32, tag="A", name=f"pp{m}")
            for j, i in enumerate(kcs):
                for n in range(2):
                    sl = bass.ts(n, 512)
                    mm(pp[:, sl], pwc[i][:, m * 128:(m + 1) * 128],
                       oTn[i][:, sl], start=(j == 0), stop=False)
            return pp

        def emit_pp_finish(m, pp, kcs, eng):
            for j, i in enumerate(kcs):
                for n in range(2):
                    sl = bass.ts(n, 512)
                    mm(pp[:, sl], pwc[i][:, m * 128:(m + 1) * 128],
                       oTn[i][:, sl], start=False, stop=(j == len(kcs) - 1))
            ou = wk.tile([128, N], BF16, tag="out", name=f"ou{m}")
            with nc.allow_low_precision(reason="partial proj sums to bf16"):
                if eng == 0:
                    nc.vector.tensor_copy(ou[:, :], pp[:, :])
                else:
                    nc.scalar.copy(ou[:, :], pp[:, :])
            dmaeng = nc.sync if eng == 0 else nc.gpsimd
            dmaeng.dma_start(out=out_dram[m * 128:(m + 1) * 128, :],
                             in_=ou[:, :])

        po_t = {}
        for h in range(4):
            po_t[h] = (ps.tile([97, 512], F32, tag="B", name=f"po{h}_0",
                               bufs=4),
                       ps.tile([97, 512], F32, tag="B", name=f"po{h}_1",
                               bufs=4))
            exs = {}
            exs[0] = emit_sc(h, 0)
            exs[1] = emit_sc(h, 1)
            for m in range(8):
                # PE fillers: head 0 chews the remaining V tiles, heads 1-2
                # the next head's q/k halves, head 3 the proj prestart
                if h == 0 and m < 6:
                    emit_v(m + 2)
                elif h in (1, 2) and m in (0, 2, 3, 5):
                    emit_qk_half(h + 1, *((0, 0) if m == 0 else
                                          (0, 1) if m == 2 else
                                          (1, 0) if m == 3 else (1, 1)))
                elif h == 3 and m == 3:
                    # prestart proj m=0 (K-chunks 0/2, head-3-free) in the
                    # two spare B half-slots as head-3 PE filler
                    pp0h = []
                    for n in range(2):
                        sl = bass.ts(n, 512)
                        pph = ps.tile([128, 512], F32, tag="B",
                                      name=f"pp0_{n}", bufs=4)
                        for j, i in enumerate([0, 2]):
                            mm(pph[:, :], pwc[i][:, 0:128],
                               oTn[i][:, sl], start=(j == 0), stop=False)
                        pp0h.append(pph)
                if m < 6:
                    exs[m + 2] = emit_sc(h, m + 2)
                if m >= 1:
                    emit_av(h, m - 1, exs[m - 1], po_t[h])
                    del exs[m - 1]
            emit_av(h, 7, exs[7], po_t[h])
            del exs[7]
            emit_po_evict(h)
            emit_recip(h)
            emit_norm(h)
            if h == 3:
                # more prestart right after the last attn@V: m=1,3 in the A
                # slots (pss rotation is over), m=2 in B once po(3) evicts
                pp_pre1 = emit_pp_start(1, [0, 2])
                pp2h = []
                for n in range(2):
                    sl = bass.ts(n, 512)
                    pph = ps.tile([128, 512], F32, tag="B",
                                  name=f"pp2_{n}", bufs=4)
                    for j, i in enumerate([0, 2]):
                        mm(pph[:, :], pwc[i][:, 2 * 128:3 * 128],
                           oTn[i][:, sl], start=(j == 0), stop=False)
                    pp2h.append(pph)

        # ---------------- proj finish ----------------
        def finish_halves(m, pphs, ou_name, eng):
            ou = wk.tile([128, N], BF16, tag="out", name=ou_name)
            for n in range(2):
                sl = bass.ts(n, 512)
                mm(pphs[n][:, :], pwc[1][:, m * 128:(m + 1) * 128],
                   oTn[1][:, sl], start=False, stop=True)
                with nc.allow_low_precision(reason="partial proj to bf16"):
                    if n == 0:
                        nc.vector.tensor_copy(ou[:, sl], pphs[n][:, :])
                    else:
                        nc.scalar.copy(ou[:, sl], pphs[n][:, :])
            dmaeng = nc.sync if eng == 0 else nc.gpsimd
            dmaeng.dma_start(out=out_dram[m * 128:(m + 1) * 128, :],
                             in_=ou[:, :])

        finish_halves(0, pp0h, "ou0", 0)
        emit_pp_finish(1, pp_pre1, [1], 1)
        finish_halves(2, pp2h, "ou2", 0)
        for m in range(3, 6):
            pp = emit_pp_start(m, [0, 2])
            emit_pp_finish(m, pp, [1], m % 2)

    _persist_ctx.close()


def _build_nc():
    nc = bacc.Bacc("TRN2", target_bir_lowering=False, debug=False,
                   enable_asserts=False)
    io = {}
    for name, shape in (("x0T", [D, N]),
                        ("qwa", [128, 768]), ("qwb", [128, 768]),
                        ("qwc", [128, 768]),
                        ("kwa", [128, 768]), ("kwb", [128, 768]),
                        ("kwc", [128, 768]), ("vw", [128, 2304]),
                        ("pw", [128, 2304]),
                        ("vones", [128, 8, 4])):
        io[name] = nc.dram_tensor(name, shape, BF16, kind="ExternalInput").ap()
    out_dram = nc.dram_tensor("o", [D, N], BF16, kind="ExternalOutput").ap()
    with tile.TileContext(nc) as tc:
        _body(nc, tc, io, out_dram)
    nc.compile()
    return nc


_NC_CACHE = {}


def _get_nc():
    if "nc" not in _NC_CACHE:
        _NC_CACHE["nc"] = _build_nc()
    return _NC_CACHE["nc"]


_HOST_STATE = {}


def _prep_in_maps(sam, conv_w, conv_b, ln_g, ln_b, pos, q_w, kv_w, proj_w,
                  proj_b):
    f = np.float32
    sam = np.asarray(sam, f)
    qwL = (np.asarray(q_w[LAYER], f) * SCALE).astype(f)
    kvL = np.asarray(kv_w[LAYER], f)
    kwL, vwL = kvL[:, :D], kvL[:, D:]
    pwL = np.asarray(proj_w[LAYER], f)
    pbL = np.asarray(proj_b[LAYER], f)

    # host patch-embed conv + LN + pos  (exact fp32; 0.4 GFLOP)
    W2 = np.asarray(conv_w, f).reshape(D, 64).T
    cb = np.asarray(conv_b, f)
    g_ = np.asarray(ln_g, f)
    b_ = np.asarray(ln_b, f)
    posf = np.asarray(pos, f)
    x0 = np.empty((B, N, D), f)
    for b in range(B):
        img = sam[b, 0]
        patches = img.reshape(32, 8, 32, 8).transpose(0, 2, 1, 3).reshape(N, 64)
        x = patches @ W2 + cb[None, :]
        mu = x.mean(-1, keepdims=True)
        var = ((x - mu) ** 2).mean(-1, keepdims=True)
        x0[b] = (x - mu) / np.sqrt(var + 1e-5) * g_ + b_ + posf
    _HOST_STATE["x0"] = x0
    _HOST_STATE["pb"] = pbL

    in_maps = []
    for c in range(8):
        b, g = c >> 1, c & 1
        sl = slice(g * 384, (g + 1) * 384)
        in_maps.append({
            "x0T": np.ascontiguousarray(x0[b].T).astype(NPBF),
            "qwa": np.ascontiguousarray(_pack6(qwL[:, sl])[:, 0:768]),
            "qwb": np.ascontiguousarray(_pack6(qwL[:, sl])[:, 768:1536]),
            "qwc": np.ascontiguousarray(_pack6(qwL[:, sl])[:, 1536:2304]),
            "kwa": np.ascontiguousarray(_pack6(kwL[:, sl])[:, 0:768]),
            "kwb": np.ascontiguousarray(_pack6(kwL[:, sl])[:, 768:1536]),
            "kwc": np.ascontiguousarray(_pack6(kwL[:, sl])[:, 1536:2304]),
            "vw": _pack6(vwL[:, sl]),
            "pw": _pack3(pwL[sl, :][_PERM, :]),
            "vones": np.ones((128, 8, 4), np.float32).astype(NPBF),
        })
    return in_maps


def _gather(results):
    x0 = _HOST_STATE["x0"]
    pb = _HOST_STATE["pb"]
    outs = [np.asarray(r["o"]).astype(np.float32) for r in results]
    full = np.stack([(outs[2 * b] + outs[2 * b + 1]).T + x0[b] + pb[None, :]
                     for b in range(B)])
    return np.ascontiguousarray(full.astype(np.float32))


def kernel(sam, conv_w, conv_b, ln_g, ln_b, pos, q_w, kv_w, proj_w, proj_b,
           **_unused):
    nc = _get_nc()
    in_maps = _prep_in_maps(sam, conv_w, conv_b, ln_g, ln_b, pos, q_w, kv_w,
                            proj_w, proj_b)
    res = run_bass_kernel_spmd(nc, in_maps, core_ids=list(range(8)))
    return _gather(res.results)


if __name__ == "__main__":
    sys.path.insert(0, os.path.dirname(os.path.abspath(__file__)))
    import reference as R

    inputs = {k: np.asarray(v) for k, v in R.setup_inputs().items()}
    expected = np.asarray(R.reference(**inputs))
    actual = kernel(**inputs)
    rel = np.linalg.norm(actual - expected) / np.linalg.norm(expected)
    print("Relative error:", rel)
